# revision 16
# baseline (speedup 1.0000x reference)
"""Trainium2 Bass kernel for the non-local-attention block (nn_DNL_74234214744693).

Reference computation (B=4, C=64, H=W=64, N=H*W=4096):
    k = conv1x1(x,kw,kb); k_wh = k - mean_j(k)
    q = conv1x1(x,qw,qb); q_wh = q - mean_j(q)
    qk[b,i,j] = sum_c k_wh[b,c,i] q_wh[b,c,j]
    m  = conv1x1(x,mw,mb) -> [B,N];  mm[b,i,j] = m[b,i]*m[b,j]
    f  = softmax(qk, axis=-1) + softmax(mm, axis=0)   # second softmax over BATCH
    y  = einsum('bci,bij->bcj', v, f) + BN(conv1x1(x,ww,wb))

Key algebraic facts used:
  * softmax_j(k_whT q_wh) == softmax_j(k_whT q_raw): q-mean cancels in softmax.
  * softmax_j normalizer Z1[i] indexes the contraction dim, so y1 = (v/Z1) @ e1.
  * batch softmax via Taylor-on-PE: exp(m_i m_j) = sum_k (m_i^k/sqrt(k!))(m_j^k/sqrt(k!)),
    a rank-K factorization evaluated as TensorE matmuls (contraction K), and the
    batch denominator D = sum_b e2_b as ONE stacked matmul (contraction 4K).
    This moves the e2 exp work off the (bottleneck) Activation engine entirely.
    D uses bf16 powers (so 1/D can never blow up); per-batch e2 uses fp8+DoubleRow.
  * sum_b f2_b == 1 exactly, so batch 3's attention is eliminated:
    y2_3 = (sum_i v_3) (x) ones - sum_{b<3} v_3 @ f2_b  (rank-1 + negated-v matmuls).

Schedule: three phases.  A: convs + m-powers (short).  B: the f1 path
(qk matmuls + ACT exp) interleaved 1:2 with f2 production units
(D/e2 matmuls on PE -> reciprocal + normalize on DVE), sharing one
ping-pong [128,2048] PSUM pool so ACT and DVE both stay saturated.
C: all attention-value matmuls (fp8 DoubleRow) + residual + output.

Sharding: each of 8 cores owns a 512-row i-slice of the [N,N] maps for ALL 4
batch samples.  Each core receives x with the j axis ROTATED by -core*512 so
its own i-slice is always columns [0,512) -- the compiled program is identical
across cores (pure SPMD), and the host un-rotates the partial outputs.
Each core emits a partial y [4,64,4096]; host sums the 8 partials.
The conv+BN residual is folded into the output matmul with weights pre-scaled
by 1/8 (so the host-side sum reconstructs it exactly once).  v is pre-scaled
by VSCALE on the host (undone host-side) to center fp8 dynamic range.
"""

import functools
import math

import numpy as np
import ml_dtypes

N_CORES = 8
B, C, H, W = 4, 64, 64, 64
N = H * W                 # 4096
SL = N // N_CORES         # 512  rows of the attention map per core
NIT = SL // 128           # 4    128-row tiles per core
NJQ = 8                   # 512-wide column blocks
JQ = N // NJQ             # 512
K = 12                    # Taylor terms for exp(m_i m_j)
KH = K // 2               # DoubleRow half-contraction
EPS = 1e-5

FP8 = True
SHIFT = 7.0 if FP8 else 0.0
VSCALE = 64.0 if FP8 else 1.0

BF16 = ml_dtypes.bfloat16


def _build_program():
    import concourse.bass as bass
    import concourse.tile as tile
    from concourse import bacc, mybir

    dt = mybir.dt
    AF = mybir.ActivationFunctionType
    ALU = mybir.AluOpType
    AX = mybir.AxisListType
    PM = mybir.MatmulPerfMode

    f8 = dt.float8e4 if FP8 else dt.bfloat16
    pm_dr = PM.DoubleRow if FP8 else None

    nc = bacc.Bacc("TRN2", target_bir_lowering=False, debug=False,
                   enable_asserts=False, num_devices=1)

    # ---------------- DRAM I/O ----------------
    x_ext = nc.dram_tensor("x_ext", [B, C + 1, N], dt.bfloat16, kind="ExternalInput")
    qmT = nc.dram_tensor("qmT", [C + 1, C + 1], dt.bfloat16, kind="ExternalInput")
    kT = nc.dram_tensor("kT", [C + 1, C], dt.bfloat16, kind="ExternalInput")
    vmT = nc.dram_tensor("vmT", [C + 1, C + 1], dt.bfloat16, kind="ExternalInput")
    wT = nc.dram_tensor("wT", [C + 1, C], dt.bfloat16, kind="ExternalInput")
    sqf = nc.dram_tensor("sqf", [1, K * 32], dt.bfloat16, kind="ExternalInput")
    y_part = nc.dram_tensor("y_part", [B, C, N], dt.float32, kind="ExternalOutput")

    NB_E2 = B - 1   # batches whose e2/f2 are computed explicitly

    with tile.TileContext(nc) as tc:
        from contextlib import ExitStack

        with ExitStack() as top:
            # ---------- persistent pools ----------
            consts = top.enter_context(tc.tile_pool(name="consts", bufs=1))
            p_kwh = top.enter_context(tc.tile_pool(name="p_kwh", bufs=B))
            p_vT = top.enter_context(tc.tile_pool(name="p_vT", bufs=B * 2))
            p_v8 = top.enter_context(tc.tile_pool(name="p_v8", bufs=B * 2 + 2))
            p_v1p = top.enter_context(tc.tile_pool(name="p_v1p", bufs=B * 2))
            p_f1 = top.enter_context(tc.tile_pool(name="p_f1", bufs=B * 2))
            p_q = top.enter_context(tc.tile_pool(name="p_q", bufs=B))
            p_P = top.enter_context(tc.tile_pool(name="p_P", bufs=1))
            p_f2 = top.enter_context(tc.tile_pool(name="p_f2", bufs=NJQ * NB_E2))
            p_small = top.enter_context(tc.tile_pool(name="p_small", bufs=B * 2 + 2))
            dram = top.enter_context(tc.tile_pool(name="dram", bufs=1, space="DRAM"))

            sb_qmT = consts.tile([C + 1, C + 1], dt.bfloat16)
            sb_kT = consts.tile([C + 1, C], dt.bfloat16)
            sb_vmT = consts.tile([C + 1, C + 1], dt.bfloat16)
            sb_wT = consts.tile([C + 1, C], dt.bfloat16)
            sqf_bc = consts.tile([128, K * 32], dt.bfloat16)
            nc.sync.dma_start(sb_qmT, qmT.ap())
            nc.sync.dma_start(sb_kT, kT.ap())
            nc.sync.dma_start(sb_vmT, vmT.ap())
            nc.sync.dma_start(sb_wT, wT.ap())
            nc.sync.dma_start(sqf_bc, sqf.ap().to_broadcast([128, K * 32]))
            biasT = None
            if FP8:
                biasT = consts.tile([128, 1], dt.float32)
                nc.vector.memset(biasT, -SHIFT)

            md = dram.tile([B, N], dt.bfloat16)
            Pd = dram.tile([B, K, N], f8)
            PdD = dram.tile([B, K, N], dt.bfloat16)

            # bf16 stacked powers for D; fp8 DoubleRow-layout per-batch powers
            # for e2 (matmul operands must start at partition base 0).
            P_all = p_P.tile([B * K, N], dt.bfloat16, name="P_all")
            if FP8:
                # three per-batch DoubleRow power blocks packed at partition
                # bases 0/32/64 (the only legal matmul base partitions)
                P_pack = p_P.tile([64 + KH, 2 * N], f8, name="P_pack")
                P_b = [P_pack[32 * b:32 * b + KH, :] for b in range(NB_E2)]
            else:
                P_pack = p_P.tile([64 + K, N], dt.bfloat16, name="P_pack")
                P_b = [P_pack[32 * b:32 * b + K, :] for b in range(NB_E2)]

            k_wh = [p_kwh.tile([C, SL], dt.bfloat16, name=f"k_wh{b}", tag="k_wh") for b in range(B)]
            q_st = [p_q.tile([C + 1, N], dt.bfloat16, name=f"q_st{b}", tag="q_st") for b in range(B)]
            v_T = [[p_vT.tile([128, 2 * C], dt.bfloat16, name=f"v_T{b}_{p}", tag="v_T") for p in range(2)] for b in range(B)]
            v_8 = [[p_v8.tile([128, 2 * C], f8, name=f"v_8{b}_{p}", tag="v_8") for p in range(2)] for b in range(B)]
            v1p = [[p_v1p.tile([128, 2 * C], f8, name=f"v1p{b}_{p}", tag="v1p") for p in range(2)] for b in range(B)]
            f1 = [[p_f1.tile([128, 2 * N], f8, name=f"f1_{b}_{p}", tag="f1") for p in range(2)] for b in range(B)]
            # f2 for all (jq, b<3): [it0|it1|it2|it3] 512-col blocks
            f2p = [[p_f2.tile([128, NIT * JQ], f8, name=f"f2p{jq}_{b}", tag="f2p")
                    for b in range(NB_E2)] for jq in range(NJQ)]
            negku = [p_small.tile([C, 1], dt.float32, name=f"negku{b}", tag="negku") for b in range(B)]
            v8n = [p_v8.tile([128, 2 * C], f8, name=f"v8n{p}", tag="v8n") for p in range(2)]
            ones_col = consts.tile([128, 1], dt.bfloat16)
            ones_row = consts.tile([1, JQ], dt.bfloat16)
            cs_row = consts.tile([1, C], dt.bfloat16)
            nc.vector.memset(ones_col, 1.0)
            nc.vector.memset(ones_row, 1.0)

            # ================= phase A: convs + powers =================
            with ExitStack() as phA:
                p_x = phA.enter_context(tc.tile_pool(name="p_x", bufs=2))
                p_xf = phA.enter_context(tc.tile_pool(name="p_xf", bufs=2))
                psA = phA.enter_context(tc.tile_pool(name="psA", bufs=2, space="PSUM"))
                p_t0 = phA.enter_context(tc.tile_pool(name="p_t0", bufs=8))
                p_pow = phA.enter_context(tc.tile_pool(name="p_pow", bufs=6))

                def dma_phase(b):
                    x_sb = p_x.tile([C + 1, N], dt.bfloat16, name=f"x_sb{b}", tag="x_sb")
                    nc.sync.dma_start(x_sb, x_ext.ap()[b])
                    return x_sb

                def xu_phase(b, x_sb):
                    t1 = p_xf.tile([C + 1, 1024], dt.bfloat16, name=f"t1_{b}", tag="t1")
                    nc.gpsimd.tensor_tensor(t1, x_sb[:, 0:1024], x_sb[:, 1024:2048], op=ALU.add)
                    nc.gpsimd.tensor_tensor(t1, t1, x_sb[:, 2048:3072], op=ALU.add)
                    nc.gpsimd.tensor_tensor(t1, t1, x_sb[:, 3072:4096], op=ALU.add)
                    nc.gpsimd.tensor_tensor(t1[:, 0:512], t1[:, 0:512], t1[:, 512:1024], op=ALU.add)
                    nc.gpsimd.tensor_tensor(t1[:, 0:256], t1[:, 0:256], t1[:, 256:512], op=ALU.add)
                    xu_f = p_t0.tile([C + 1, 1], dt.float32, tag="t0")
                    xu_bf = p_t0.tile([C + 1, 1], dt.bfloat16, tag="t0b")
                    nc.vector.tensor_reduce(xu_f, t1[:, 0:256], axis=AX.X, op=ALU.add)
                    nc.vector.tensor_scalar_mul(xu_bf, xu_f, 1.0 / N)
                    return xu_bf

                def conv_phase(b, x_sb, xu_bf):
                    # ku + v/m convs + k conv packed into one psum tile
                    ps_m = psA.tile([128, 2048], dt.float32, name=f"ps_m{b}", tag="psA")
                    nc.tensor.matmul(ps_m[0:C, 1536:1537], sb_kT, xu_bf,
                                     start=True, stop=True)
                    nc.vector.tensor_scalar_mul(negku[b], ps_m[0:C, 1536:1537], -1.0)
                    for it in range(NIT):
                        fo = (it // 2) * 512 + (it % 2) * 256
                        nc.tensor.matmul(ps_m[:, fo:fo + C + 1],
                                         x_sb[:, it * 128:(it + 1) * 128],
                                         sb_vmT, start=True, stop=True)
                    nc.tensor.matmul(ps_m[0:C, 1024:1536], sb_kT, x_sb[:, 0:SL],
                                     start=True, stop=True)
                    for it in range(NIT):
                        fo = (it // 2) * 512 + (it % 2) * 256
                        vt_dst = v_T[b][it // 2][:, (it % 2) * C:(it % 2) * C + C]
                        nc.vector.tensor_copy(vt_dst, ps_m[:, fo:fo + C])
                        v8_dst = v_8[b][it // 2][:, (it % 2) * C:(it % 2) * C + C]
                        nc.gpsimd.tensor_copy(v8_dst, vt_dst)
                        if b == B - 1:
                            nc.gpsimd.tensor_scalar_mul(
                                v8n[it // 2][:, (it % 2) * C:(it % 2) * C + C],
                                vt_dst, -1.0)
                    nc.vector.tensor_scalar(k_wh[b], ps_m[0:C, 1024:1536],
                                            scalar1=negku[b], scalar2=None, op0=ALU.add)

                    # q conv (row 64 = m)
                    for half in range(2):
                        ps_q = psA.tile([128, 2048], dt.float32, name=f"ps_q{b}_{half}", tag="psA")
                        for k4 in range(4):
                            j0 = half * 2048 + k4 * 512
                            nc.tensor.matmul(ps_q[0:C + 1, k4 * 512:(k4 + 1) * 512],
                                             sb_qmT, x_sb[:, j0:j0 + 512],
                                             start=True, stop=True)
                        dst = q_st[b][:, half * 2048:(half + 1) * 2048]
                        if half == 0:
                            nc.vector.tensor_copy(dst, ps_q[0:C + 1, :])
                        else:
                            nc.scalar.copy(dst, ps_q[0:C + 1, :])
                    nc.sync.dma_start(md[b], q_st[b][C:C + 1, :])

                def powers_phase(b):
                    pw = p_pow.tile([128, K * 32], dt.bfloat16, name=f"pw{b}", tag="pw")
                    pwsb = p_pow.tile([128, K * 32], dt.bfloat16, name=f"pwsb{b}", tag="pwsb")
                    nc.gpsimd.memset(pw[:, 0:32], 1.0)
                    nc.sync.dma_start(pw[:, 32:64],
                                      md[b:b + 1, :].rearrange("o (p c) -> (o p) c", p=128))
                    for kk in range(2, K):
                        nc.gpsimd.tensor_tensor(pw[:, kk * 32:(kk + 1) * 32],
                                                pw[:, (kk - 1) * 32:kk * 32],
                                                pw[:, 32:64], op=ALU.mult)
                    nc.gpsimd.tensor_tensor(pwsb, pw, sqf_bc, op=ALU.mult)
                    nc.sync.dma_start(PdD[b].rearrange("kk (p c) -> p kk c", p=128), pwsb)
                    nc.sync.dma_start(P_all[K * b:K * (b + 1), :], PdD[b])
                    if b < NB_E2:
                        if FP8:
                            pws = p_pow.tile([128, K * 32], f8, name=f"pws{b}", tag="pws")
                            nc.gpsimd.tensor_copy(pws, pwsb)
                            nc.sync.dma_start(Pd[b].rearrange("kk (p c) -> p kk c", p=128), pws)
                            nc.sync.dma_start(P_b[b],
                                              Pd[b].rearrange("(i r) j -> r i j", i=2))
                        else:
                            nc.sync.dma_start(P_b[b], PdD[b])

                x_cur = dma_phase(0)
                xu_cur = xu_phase(0, x_cur)
                for b in range(B):
                    if b + 1 < B:
                        x_next = dma_phase(b + 1)
                        xu_next = xu_phase(b + 1, x_next)
                    conv_phase(b, x_cur, xu_cur)
                    powers_phase(b)
                    if b + 1 < B:
                        x_cur, xu_cur = x_next, xu_next

            # ===== phase B: f1 (qk+exp) interleaved with f2 production =====
            with ExitStack() as phB:
                psP = phB.enter_context(tc.tile_pool(name="psP", bufs=2, space="PSUM"))
                p_z = phB.enter_context(tc.tile_pool(name="p_z", bufs=8))
                p_rr = phB.enter_context(tc.tile_pool(name="p_rr", bufs=3))

                if FP8:
                    P3b = [P_b[b].rearrange("p (i j) -> p i j", i=2) for b in range(NB_E2)]

                def qk_unit(b, it):
                    zp = [p_z.tile([128, 1], dt.float32, name=f"zp{j}", tag="zp") for j in range(2)]
                    for jh in range(2):
                        ps_qk = psP.tile([128, 2048], dt.float32, name="ps_qk", tag="psP")
                        for k4 in range(4):
                            j0 = jh * 2048 + k4 * 512
                            nc.tensor.matmul(
                                ps_qk[:, k4 * 512:(k4 + 1) * 512],
                                k_wh[b][:, it * 128:(it + 1) * 128],
                                q_st[b][0:C, j0:j0 + 512],
                                start=True, stop=True)
                        dst = f1[b][it // 2][:, (it % 2) * N + jh * 2048:
                                             (it % 2) * N + (jh + 1) * 2048]
                        if FP8:
                            nc.scalar.activation(dst, ps_qk, AF.Exp,
                                                 bias=biasT[:, 0:1], accum_out=zp[jh])
                        else:
                            nc.scalar.activation(dst, ps_qk, AF.Exp, accum_out=zp[jh])
                    z1 = p_z.tile([128, 1], dt.float32)
                    rz = p_z.tile([128, 1], dt.float32)
                    nc.vector.tensor_tensor(z1, zp[0], zp[1], op=ALU.add)
                    nc.vector.reciprocal_approx_fast(rz, z1)
                    nc.gpsimd.tensor_scalar_mul(
                        v1p[b][it // 2][:, (it % 2) * C:(it % 2) * C + C],
                        v_T[b][it // 2][:, (it % 2) * C:(it % 2) * C + C], rz)

                def f2_unit(it, jq):
                    isl = slice(it * 128, (it + 1) * 128)
                    jsl = slice(jq * JQ, (jq + 1) * JQ)
                    ps = psP.tile([128, 2048], dt.float32, name="ps_f2", tag="psP")
                    nc.tensor.matmul(ps[:, 0:JQ], P_all[:, isl], P_all[:, jsl],
                                     start=True, stop=True)
                    for b in range(NB_E2):
                        cs = slice((b + 1) * JQ, (b + 2) * JQ)
                        if FP8:
                            nc.tensor.matmul(ps[:, cs], P3b[b][:, :, isl], P3b[b][:, :, jsl],
                                             start=True, stop=True, perf_mode=pm_dr)
                        else:
                            nc.tensor.matmul(ps[:, cs], P_b[b][:, isl], P_b[b][:, jsl],
                                             start=True, stop=True)
                    rr = p_rr.tile([128, JQ], dt.float32, name="rr", tag="rr")
                    nc.vector.reciprocal_approx_fast(rr, ps[:, 0:JQ])
                    for b in range(NB_E2):
                        cs = slice((b + 1) * JQ, (b + 2) * JQ)
                        nc.vector.tensor_tensor(
                            f2p[jq][b][:, it * JQ:(it + 1) * JQ], ps[:, cs], rr,
                            op=ALU.mult)

                fq = [(it, jq) for it in range(NIT) for jq in range(NJQ)]
                fi = 0
                for b in range(B):
                    for it in range(NIT):
                        qk_unit(b, it)
                        for _ in range(2):
                            if fi < len(fq):
                                f2_unit(*fq[fi])
                                fi += 1
                while fi < len(fq):
                    f2_unit(*fq[fi])
                    fi += 1

            # ========= phase C: attention-value matmuls + residual =========
            with ExitStack() as phC:
                psY = phC.enter_context(tc.tile_pool(name="psY", bufs=4, space="PSUM"))
                p_xw = phC.enter_context(tc.tile_pool(name="p_xw", bufs=5))
                p_out = phC.enter_context(tc.tile_pool(name="p_out", bufs=4))

                ps_cs = psY.tile([1, C], dt.float32, name="ps_cs", tag="ps_cs")
                for pr in range(2):
                    for ip in range(2):
                        nc.tensor.matmul(ps_cs, ones_col,
                                         v_T[B - 1][pr][:, ip * C:(ip + 1) * C],
                                         start=(pr == 0 and ip == 0),
                                         stop=(pr == 1 and ip == 1))
                nc.vector.tensor_copy(cs_row, ps_cs)

                for jq in range(NJQ):
                    jsl = slice(jq * JQ, (jq + 1) * JQ)
                    x_wx = []
                    for b in range(B):
                        t = p_xw.tile([C + 1, JQ], dt.bfloat16, name="x_wx", tag="x_wx")
                        nc.sync.dma_start(t, x_ext.ap()[b][:, jsl])
                        x_wx.append(t)

                    ps_y = [psY.tile([C, JQ], dt.float32, name=f"ps_y{b}", tag="ps_y")
                            for b in range(B)]
                    for b in range(B):
                        nc.tensor.matmul(ps_y[b], sb_wT, x_wx[b],
                                         start=True, stop=False)
                    nc.tensor.matmul(ps_y[B - 1], cs_row, ones_row,
                                     start=False, stop=False)

                    for pr in range(2):
                        last = (pr == 1)
                        if FP8:
                            for b in range(B):
                                nc.tensor.matmul(
                                    ps_y[b],
                                    v1p[b][pr][:, :].rearrange("p (i c) -> p i c", i=2),
                                    f1[b][pr][:, :].rearrange("p (i j) -> p i j", i=2)[:, :, jsl],
                                    start=False, stop=(last and b == B - 1 and False) or False,
                                    perf_mode=pm_dr)
                            for b in range(NB_E2):
                                f2r = f2p[jq][b][:, pr * 2 * JQ:(pr + 1) * 2 * JQ] \
                                    .rearrange("p (i j) -> p i j", i=2)
                                nc.tensor.matmul(
                                    ps_y[b],
                                    v_8[b][pr][:, :].rearrange("p (i c) -> p i c", i=2),
                                    f2r,
                                    start=False, stop=last, perf_mode=pm_dr)
                                nc.tensor.matmul(
                                    ps_y[B - 1],
                                    v8n[pr][:, :].rearrange("p (i c) -> p i c", i=2),
                                    f2r,
                                    start=False, stop=(last and b == NB_E2 - 1),
                                    perf_mode=pm_dr)
                        else:
                            for ip in range(2):
                                it = pr * 2 + ip
                                for b in range(B):
                                    nc.tensor.matmul(
                                        ps_y[b], v1p[b][pr][:, ip * C:(ip + 1) * C],
                                        f1[b][pr][:, ip * N + jq * JQ:ip * N + (jq + 1) * JQ],
                                        start=False, stop=False)
                                for b in range(NB_E2):
                                    f2_src = f2p[jq][b][:, it * JQ:(it + 1) * JQ]
                                    nc.tensor.matmul(
                                        ps_y[b], v_8[b][pr][:, ip * C:(ip + 1) * C],
                                        f2_src,
                                        start=False, stop=(last and ip == 1))
                                    nc.tensor.matmul(
                                        ps_y[B - 1], v8n[pr][:, ip * C:(ip + 1) * C],
                                        f2_src,
                                        start=False,
                                        stop=(last and ip == 1 and b == NB_E2 - 1))

                    # close the y1 groups for FP8 (stop flags handled above for f2)
                    for b in range(B):
                        out_sb = p_out.tile([C, JQ], dt.float32)
                        if b < 3:
                            nc.scalar.copy(out_sb, ps_y[b])
                        else:
                            nc.vector.tensor_copy(out_sb, ps_y[b])
                        nc.sync.dma_start(y_part.ap()[b][:, jsl], out_sb)

    nc.compile()
    return nc


@functools.lru_cache(maxsize=1)
def _get_program():
    return _build_program()


def _prep_inputs(inputs):
    x = np.asarray(inputs["x"], np.float32).reshape(B, C, N)
    ones = np.ones((B, 1, N), np.float32)
    x_ext = np.concatenate([x, ones], axis=1).astype(BF16)          # [B,65,N]

    qw = np.asarray(inputs["qw"], np.float32)
    qb = np.asarray(inputs["qb"], np.float32)
    kw = np.asarray(inputs["kw"], np.float32)
    kb = np.asarray(inputs["kb"], np.float32)
    mw = np.asarray(inputs["mw"], np.float32)
    mb = np.asarray(inputs["mb"], np.float32)
    vw = np.asarray(inputs["vw"], np.float32)
    vb = np.asarray(inputs["vb"], np.float32)
    ww = np.asarray(inputs["ww"], np.float32)
    wb = np.asarray(inputs["wb"], np.float32)
    g = np.asarray(inputs["bn_gamma"], np.float32)
    be = np.asarray(inputs["bn_beta"], np.float32)
    rm = np.asarray(inputs["bn_rm"], np.float32)
    rv = np.asarray(inputs["bn_rv"], np.float32)

    qmT = np.zeros((C + 1, C + 1), np.float32)
    qmT[:C, :C] = qw.T
    qmT[C, :C] = qb
    qmT[:C, C] = mw[0]
    qmT[C, C] = mb[0]

    kT = np.concatenate([kw.T, kb[None, :]], axis=0)                # [65,64]

    vmT = np.zeros((C + 1, C + 1), np.float32)
    vmT[:C, :C] = vw.T * VSCALE
    vmT[C, :C] = vb * VSCALE
    vmT[:C, C] = mw[0]
    vmT[C, C] = mb[0]

    inv = g / np.sqrt(rv + EPS)
    wT = np.zeros((C + 1, C), np.float32)
    wT[:C, :] = (ww * inv[:, None]).T * (VSCALE / N_CORES)
    wT[C, :] = (wb * inv + be - rm * inv) * (VSCALE / N_CORES)

    sqf = np.zeros((1, K * 32), np.float32)
    for kk in range(K):
        sqf[0, kk * 32:(kk + 1) * 32] = 1.0 / math.sqrt(math.factorial(kk))

    common = {
        "qmT": qmT.astype(BF16),
        "kT": kT.astype(BF16),
        "vmT": vmT.astype(BF16),
        "wT": wT.astype(BF16),
        "sqf": sqf.astype(BF16),
    }
    in_maps = []
    for ic in range(N_CORES):
        m = dict(common)
        m["x_ext"] = np.ascontiguousarray(np.roll(x_ext, -ic * SL, axis=2))
        in_maps.append(m)
    return in_maps


def kernel(**inputs):
    from concourse.bass_utils import run_bass_kernel_spmd

    nc = _get_program()
    in_maps = _prep_inputs(inputs)
    res = run_bass_kernel_spmd(nc, in_maps, core_ids=list(range(N_CORES)))
    y = np.zeros((B, C, N), np.float32)
    for ic, r in enumerate(res.results):
        y += np.roll(r["y_part"], ic * SL, axis=2)
    y *= 1.0 / VSCALE
    return y.reshape(B, C, H, W)


if __name__ == "__main__":
    rng = np.random.default_rng(0)
    ins = {
        "x": rng.standard_normal((B, C, H, W), dtype=np.float32),
        "qw": rng.standard_normal((C, C), dtype=np.float32) * 0.05,
        "qb": rng.standard_normal((C,), dtype=np.float32) * 0.05,
        "kw": rng.standard_normal((C, C), dtype=np.float32) * 0.05,
        "kb": rng.standard_normal((C,), dtype=np.float32) * 0.05,
        "mw": rng.standard_normal((1, C), dtype=np.float32) * 0.05,
        "mb": rng.standard_normal((1,), dtype=np.float32) * 0.05,
        "vw": rng.standard_normal((C, C), dtype=np.float32) * 0.05,
        "vb": rng.standard_normal((C,), dtype=np.float32) * 0.05,
        "ww": rng.standard_normal((C, C), dtype=np.float32) * 0.05,
        "wb": rng.standard_normal((C,), dtype=np.float32) * 0.05,
        "bn_gamma": np.ones((C,), np.float32),
        "bn_beta": np.zeros((C,), np.float32),
        "bn_rm": np.zeros((C,), np.float32),
        "bn_rv": np.ones((C,), np.float32),
    }
    out = kernel(**ins)
    print("kernel output", out.shape, out.dtype, np.abs(out).mean())


# revision 19
# speedup vs baseline: 1.2404x; 1.2404x over previous
"""Trainium2 Bass kernel for the non-local-attention block (nn_DNL_74234214744693).

Reference computation (B=4, C=64, H=W=64, N=H*W=4096):
    k = conv1x1(x,kw,kb); k_wh = k - mean_j(k)
    q = conv1x1(x,qw,qb); q_wh = q - mean_j(q)
    qk[b,i,j] = sum_c k_wh[b,c,i] q_wh[b,c,j]
    m  = conv1x1(x,mw,mb) -> [B,N];  mm[b,i,j] = m[b,i]*m[b,j]
    f  = softmax(qk, axis=-1) + softmax(mm, axis=0)   # second softmax over BATCH
    y  = einsum('bci,bij->bcj', v, f) + BN(conv1x1(x,ww,wb))

Key algebraic facts used:
  * softmax_j(k_whT q_wh) == softmax_j(k_whT q_raw): q-mean cancels in softmax.
  * softmax_j normalizer Z1[i] indexes the contraction dim, so y1 = (v/Z1) @ e1.
  * batch softmax via Taylor-on-PE: exp(m_i m_j) = sum_k (m_i^k/sqrt(k!))(m_j^k/sqrt(k!)),
    a rank-K factorization evaluated as TensorE matmuls (contraction K), and the
    batch denominator D = sum_b e2_b as ONE stacked matmul (contraction 4K).
    This moves the e2 exp work off the (bottleneck) Activation engine entirely.
    D uses bf16 powers (so 1/D can never blow up); per-batch e2 uses fp8+DoubleRow.
  * sum_b f2_b == 1 exactly, so batch 3's attention is eliminated:
    y2_3 = (sum_i v_3) (x) ones - sum_{b<3} v_3 @ f2_b  (rank-1 + negated-v matmuls).

Schedule: three phases.  A: convs + m-powers (short).  B: the f1 path
(qk matmuls + ACT exp) interleaved 1:2 with f2 production units
(D/e2 matmuls on PE -> reciprocal + normalize on DVE), sharing one
ping-pong [128,2048] PSUM pool so ACT and DVE both stay saturated.
C: all attention-value matmuls (fp8 DoubleRow) + residual + output.

Sharding: each of 8 cores owns a 512-row i-slice of the [N,N] maps for ALL 4
batch samples.  Each core receives x with the j axis ROTATED by -core*512 so
its own i-slice is always columns [0,512) -- the compiled program is identical
across cores (pure SPMD), and the host un-rotates the partial outputs.
Each core emits a partial y [4,64,4096]; host sums the 8 partials.
The conv+BN residual is folded into the output matmul with weights pre-scaled
by 1/8 (so the host-side sum reconstructs it exactly once).  v is pre-scaled
by VSCALE on the host (undone host-side) to center fp8 dynamic range.
"""

import functools
import math

import numpy as np
import ml_dtypes

N_CORES = 8
B, C, H, W = 4, 64, 64, 64
N = H * W                 # 4096
SL = N // N_CORES         # 512  rows of the attention map per core
NIT = SL // 128           # 4    128-row tiles per core
NJQ = 8                   # 512-wide column blocks
JQ = N // NJQ             # 512
K = 12                    # Taylor terms for exp(m_i m_j)
KH = K // 2               # DoubleRow half-contraction
EPS = 1e-5

FP8 = True
SHIFT = 7.0 if FP8 else 0.0
VSCALE = 64.0 if FP8 else 1.0

BF16 = ml_dtypes.bfloat16


def _build_program():
    import concourse.bass as bass
    import concourse.tile as tile
    from concourse import bacc, mybir

    dt = mybir.dt
    AF = mybir.ActivationFunctionType
    ALU = mybir.AluOpType
    AX = mybir.AxisListType
    PM = mybir.MatmulPerfMode

    f8 = dt.float8e4 if FP8 else dt.bfloat16
    pm_dr = PM.DoubleRow if FP8 else None

    nc = bacc.Bacc("TRN2", target_bir_lowering=False, debug=False,
                   enable_asserts=False, num_devices=1)

    # ---------------- DRAM I/O ----------------
    x_ext = nc.dram_tensor("x_ext", [B, C + 1, N], dt.bfloat16, kind="ExternalInput")
    qmT = nc.dram_tensor("qmT", [C + 1, C + 1], dt.bfloat16, kind="ExternalInput")
    kT = nc.dram_tensor("kT", [C + 1, C], dt.bfloat16, kind="ExternalInput")
    vmT = nc.dram_tensor("vmT", [C + 1, C + 1], dt.bfloat16, kind="ExternalInput")
    wT = nc.dram_tensor("wT", [C + 1, C], dt.bfloat16, kind="ExternalInput")
    sqf = nc.dram_tensor("sqf", [1, K * 32], dt.bfloat16, kind="ExternalInput")
    y_part = nc.dram_tensor("y_part", [B, C, N], dt.float32, kind="ExternalOutput")

    NB_E2 = B - 1   # batches whose e2/f2 are computed explicitly

    with tile.TileContext(nc) as tc:
        from contextlib import ExitStack

        with ExitStack() as top:
            # ---------- persistent pools ----------
            consts = top.enter_context(tc.tile_pool(name="consts", bufs=1))
            p_kwh = top.enter_context(tc.tile_pool(name="p_kwh", bufs=B))
            p_vT = top.enter_context(tc.tile_pool(name="p_vT", bufs=B * 2))
            p_v8 = top.enter_context(tc.tile_pool(name="p_v8", bufs=B * 2 + 2))
            p_v1p = top.enter_context(tc.tile_pool(name="p_v1p", bufs=B * 2))
            p_f1 = top.enter_context(tc.tile_pool(name="p_f1", bufs=B * 2))
            p_q = top.enter_context(tc.tile_pool(name="p_q", bufs=B))
            p_P = top.enter_context(tc.tile_pool(name="p_P", bufs=1))
            p_f2 = top.enter_context(tc.tile_pool(name="p_f2", bufs=NJQ * NB_E2))
            p_small = top.enter_context(tc.tile_pool(name="p_small", bufs=B * 2 + 2))
            dram = top.enter_context(tc.tile_pool(name="dram", bufs=1, space="DRAM"))

            sb_qmT = consts.tile([C + 1, C + 1], dt.bfloat16)
            sb_kT = consts.tile([C + 1, C], dt.bfloat16)
            sb_vmT = consts.tile([C + 1, C + 1], dt.bfloat16)
            sb_wT = consts.tile([C + 1, C], dt.bfloat16)
            sqf_bc = consts.tile([128, K * 32], dt.bfloat16)
            nc.sync.dma_start(sb_qmT, qmT.ap())
            nc.sync.dma_start(sb_kT, kT.ap())
            nc.sync.dma_start(sb_vmT, vmT.ap())
            nc.sync.dma_start(sb_wT, wT.ap())
            nc.sync.dma_start(sqf_bc, sqf.ap().to_broadcast([128, K * 32]))
            biasT = None
            if FP8:
                biasT = consts.tile([128, 1], dt.float32)
                nc.vector.memset(biasT, -SHIFT)

            md = dram.tile([B, N], dt.bfloat16)
            Pd = dram.tile([B, K, N], f8)
            PdD = dram.tile([B, K, N], dt.bfloat16)

            # bf16 stacked powers for D; fp8 DoubleRow-layout per-batch powers
            # for e2 (matmul operands must start at partition base 0).
            P_all = p_P.tile([B * K, N], dt.bfloat16, name="P_all")
            if FP8:
                # three per-batch DoubleRow power blocks packed at partition
                # bases 0/32/64 (the only legal matmul base partitions)
                P_pack = p_P.tile([64 + KH, 2 * N], f8, name="P_pack")
                P_b = [P_pack[32 * b:32 * b + KH, :] for b in range(NB_E2)]
            else:
                P_pack = p_P.tile([64 + K, N], dt.bfloat16, name="P_pack")
                P_b = [P_pack[32 * b:32 * b + K, :] for b in range(NB_E2)]

            k_wh = [p_kwh.tile([C, SL], dt.bfloat16, name=f"k_wh{b}", tag="k_wh") for b in range(B)]
            q_st = [p_q.tile([C + 1, N], dt.bfloat16, name=f"q_st{b}", tag="q_st") for b in range(B)]
            v_T = [[p_vT.tile([128, 2 * C], dt.bfloat16, name=f"v_T{b}_{p}", tag="v_T") for p in range(2)] for b in range(B)]
            v_8 = [[p_v8.tile([128, 2 * C], f8, name=f"v_8{b}_{p}", tag="v_8") for p in range(2)] for b in range(B)]
            v1p = [[p_v1p.tile([128, 2 * C], f8, name=f"v1p{b}_{p}", tag="v1p") for p in range(2)] for b in range(B)]
            f1 = [[p_f1.tile([128, 2 * N], f8, name=f"f1_{b}_{p}", tag="f1") for p in range(2)] for b in range(B)]
            # f2 for all (jq, b<3): [it0|it1|it2|it3] 512-col blocks
            f2p = [[p_f2.tile([128, NIT * JQ], f8, name=f"f2p{jq}_{b}", tag="f2p")
                    for b in range(NB_E2)] for jq in range(NJQ)]
            negku = [p_small.tile([C, 1], dt.float32, name=f"negku{b}", tag="negku") for b in range(B)]
            v8n = [p_v8.tile([128, 2 * C], f8, name=f"v8n{p}", tag="v8n") for p in range(2)]
            ones_col = consts.tile([128, 1], dt.bfloat16)
            ones_row = consts.tile([1, JQ], dt.bfloat16)
            cs_row = consts.tile([1, C], dt.bfloat16)
            nc.vector.memset(ones_col, 1.0)
            nc.vector.memset(ones_row, 1.0)

            # ================= phase A: convs + powers =================
            with ExitStack() as phA:
                p_x = phA.enter_context(tc.tile_pool(name="p_x", bufs=2))
                p_xf = phA.enter_context(tc.tile_pool(name="p_xf", bufs=2))
                psA = phA.enter_context(tc.tile_pool(name="psA", bufs=2, space="PSUM"))
                p_t0 = phA.enter_context(tc.tile_pool(name="p_t0", bufs=8))
                p_pow = phA.enter_context(tc.tile_pool(name="p_pow", bufs=6))

                def dma_phase(b):
                    x_sb = p_x.tile([C + 1, N], dt.bfloat16, name=f"x_sb{b}", tag="x_sb")
                    nc.sync.dma_start(x_sb, x_ext.ap()[b])
                    return x_sb

                def xu_phase(b, x_sb):
                    t1 = p_xf.tile([C + 1, 1024], dt.bfloat16, name=f"t1_{b}", tag="t1")
                    nc.vector.tensor_tensor(t1, x_sb[:, 0:1024], x_sb[:, 1024:2048], op=ALU.add)
                    nc.vector.tensor_tensor(t1, t1, x_sb[:, 2048:3072], op=ALU.add)
                    nc.vector.tensor_tensor(t1, t1, x_sb[:, 3072:4096], op=ALU.add)
                    nc.vector.tensor_tensor(t1[:, 0:512], t1[:, 0:512], t1[:, 512:1024], op=ALU.add)
                    nc.vector.tensor_tensor(t1[:, 0:256], t1[:, 0:256], t1[:, 256:512], op=ALU.add)
                    xu_f = p_t0.tile([C + 1, 1], dt.float32, tag="t0")
                    xu_bf = p_t0.tile([C + 1, 1], dt.bfloat16, tag="t0b")
                    nc.vector.tensor_reduce(xu_f, t1[:, 0:256], axis=AX.X, op=ALU.add)
                    nc.vector.tensor_scalar_mul(xu_bf, xu_f, 1.0 / N)
                    return xu_bf

                def conv_phase(b, x_sb, xu_bf):
                    # ku + v/m convs + k conv packed into one psum tile
                    ps_m = psA.tile([128, 2048], dt.float32, name=f"ps_m{b}", tag="psA")
                    nc.tensor.matmul(ps_m[0:C, 1536:1537], sb_kT, xu_bf,
                                     start=True, stop=True)
                    nc.vector.tensor_scalar_mul(negku[b], ps_m[0:C, 1536:1537], -1.0)
                    for it in range(NIT):
                        fo = (it // 2) * 512 + (it % 2) * 256
                        nc.tensor.matmul(ps_m[:, fo:fo + C + 1],
                                         x_sb[:, it * 128:(it + 1) * 128],
                                         sb_vmT, start=True, stop=True)
                    nc.tensor.matmul(ps_m[0:C, 1024:1536], sb_kT, x_sb[:, 0:SL],
                                     start=True, stop=True)
                    for it in range(NIT):
                        fo = (it // 2) * 512 + (it % 2) * 256
                        vt_dst = v_T[b][it // 2][:, (it % 2) * C:(it % 2) * C + C]
                        nc.vector.tensor_copy(vt_dst, ps_m[:, fo:fo + C])
                        v8_dst = v_8[b][it // 2][:, (it % 2) * C:(it % 2) * C + C]
                        nc.gpsimd.tensor_copy(v8_dst, vt_dst)
                        if b == B - 1:
                            nc.gpsimd.tensor_scalar_mul(
                                v8n[it // 2][:, (it % 2) * C:(it % 2) * C + C],
                                vt_dst, -1.0)
                    nc.vector.tensor_scalar(k_wh[b], ps_m[0:C, 1024:1536],
                                            scalar1=negku[b], scalar2=None, op0=ALU.add)

                    # q conv (row 64 = m)
                    for half in range(2):
                        ps_q = psA.tile([128, 2048], dt.float32, name=f"ps_q{b}_{half}", tag="psA")
                        for k4 in range(4):
                            j0 = half * 2048 + k4 * 512
                            nc.tensor.matmul(ps_q[0:C + 1, k4 * 512:(k4 + 1) * 512],
                                             sb_qmT, x_sb[:, j0:j0 + 512],
                                             start=True, stop=True)
                        dst = q_st[b][:, half * 2048:(half + 1) * 2048]
                        if half == 0:
                            nc.vector.tensor_copy(dst, ps_q[0:C + 1, :])
                        else:
                            nc.scalar.copy(dst, ps_q[0:C + 1, :])
                    nc.sync.dma_start(md[b], q_st[b][C:C + 1, :])

                def powers_phase(b):
                    pw = p_pow.tile([128, K * 32], dt.bfloat16, name=f"pw{b}", tag="pw")
                    pwsb = p_pow.tile([128, K * 32], dt.bfloat16, name=f"pwsb{b}", tag="pwsb")
                    nc.gpsimd.memset(pw[:, 0:32], 1.0)
                    nc.sync.dma_start(pw[:, 32:64],
                                      md[b:b + 1, :].rearrange("o (p c) -> (o p) c", p=128))
                    for kk in range(2, K):
                        nc.gpsimd.tensor_tensor(pw[:, kk * 32:(kk + 1) * 32],
                                                pw[:, (kk - 1) * 32:kk * 32],
                                                pw[:, 32:64], op=ALU.mult)
                    nc.gpsimd.tensor_tensor(pwsb, pw, sqf_bc, op=ALU.mult)
                    nc.sync.dma_start(PdD[b].rearrange("kk (p c) -> p kk c", p=128), pwsb)
                    nc.sync.dma_start(P_all[K * b:K * (b + 1), :], PdD[b])
                    if b < NB_E2:
                        if FP8:
                            pws = p_pow.tile([128, K * 32], f8, name=f"pws{b}", tag="pws")
                            nc.gpsimd.tensor_copy(pws, pwsb)
                            nc.sync.dma_start(Pd[b].rearrange("kk (p c) -> p kk c", p=128), pws)
                            nc.sync.dma_start(P_b[b],
                                              Pd[b].rearrange("(i r) j -> r i j", i=2))
                        else:
                            nc.sync.dma_start(P_b[b], PdD[b])

                x_cur = dma_phase(0)
                xu_cur = xu_phase(0, x_cur)
                for b in range(B):
                    if b + 1 < B:
                        x_next = dma_phase(b + 1)
                        xu_next = xu_phase(b + 1, x_next)
                    conv_phase(b, x_cur, xu_cur)
                    powers_phase(b)
                    if b + 1 < B:
                        x_cur, xu_cur = x_next, xu_next

            # ===== phase B: f1 (qk+exp) interleaved with f2 production =====
            with ExitStack() as phB:
                psP = phB.enter_context(tc.tile_pool(name="psP", bufs=4, space="PSUM"))
                p_z = phB.enter_context(tc.tile_pool(name="p_z", bufs=8))
                p_rr = phB.enter_context(tc.tile_pool(name="p_rr", bufs=3))

                if FP8:
                    P3b = [P_b[b].rearrange("p (i j) -> p i j", i=2) for b in range(NB_E2)]

                def qk_unit(b, it):
                    zp = [p_z.tile([128, 1], dt.float32, name=f"zp{j}", tag="zp") for j in range(4)]
                    for q4 in range(4):
                        ps_qk = psP.tile([128, 1024], dt.float32, name="ps_qk", tag="psP")
                        for k2 in range(2):
                            j0 = q4 * 1024 + k2 * 512
                            nc.tensor.matmul(
                                ps_qk[:, k2 * 512:(k2 + 1) * 512],
                                k_wh[b][:, it * 128:(it + 1) * 128],
                                q_st[b][0:C, j0:j0 + 512],
                                start=True, stop=True)
                        dst = f1[b][it // 2][:, (it % 2) * N + q4 * 1024:
                                             (it % 2) * N + (q4 + 1) * 1024]
                        if FP8:
                            nc.scalar.activation(dst, ps_qk, AF.Exp,
                                                 bias=biasT[:, 0:1], accum_out=zp[q4])
                        else:
                            nc.scalar.activation(dst, ps_qk, AF.Exp, accum_out=zp[q4])
                    z1 = p_z.tile([128, 1], dt.float32)
                    rz = p_z.tile([128, 1], dt.float32)
                    nc.vector.tensor_tensor(z1, zp[0], zp[1], op=ALU.add)
                    nc.vector.tensor_tensor(zp[2], zp[2], zp[3], op=ALU.add)
                    nc.vector.tensor_tensor(z1, z1, zp[2], op=ALU.add)
                    nc.vector.reciprocal_approx_fast(rz, z1)
                    nc.gpsimd.tensor_scalar_mul(
                        v1p[b][it // 2][:, (it % 2) * C:(it % 2) * C + C],
                        v_T[b][it // 2][:, (it % 2) * C:(it % 2) * C + C], rz)

                def f2_unit(it, jq):
                    isl = slice(it * 128, (it + 1) * 128)
                    jsl = slice(jq * JQ, (jq + 1) * JQ)
                    t0 = psP.tile([128, 1024], dt.float32, name="ps_f2a", tag="psP")
                    t1 = psP.tile([128, 1024], dt.float32, name="ps_f2b", tag="psP")
                    nc.tensor.matmul(t0[:, 0:JQ], P_all[:, isl], P_all[:, jsl],
                                     start=True, stop=True)
                    e2dst = [t0[:, JQ:2 * JQ], t1[:, 0:JQ], t1[:, JQ:2 * JQ]]
                    for b in range(NB_E2):
                        if FP8:
                            nc.tensor.matmul(e2dst[b], P3b[b][:, :, isl], P3b[b][:, :, jsl],
                                             start=True, stop=True, perf_mode=pm_dr)
                        else:
                            nc.tensor.matmul(e2dst[b], P_b[b][:, isl], P_b[b][:, jsl],
                                             start=True, stop=True)
                    rr = p_rr.tile([128, JQ], dt.float32, name="rr", tag="rr")
                    nc.vector.reciprocal_approx_fast(rr, t0[:, 0:JQ])
                    for b in range(NB_E2):
                        nc.vector.tensor_tensor(
                            f2p[jq][b][:, it * JQ:(it + 1) * JQ], e2dst[b], rr,
                            op=ALU.mult)

                fq = [(it, jq) for it in range(NIT) for jq in range(NJQ)]
                fi = 0
                for b in range(B):
                    for it in range(NIT):
                        qk_unit(b, it)
                        for _ in range(2):
                            if fi < len(fq):
                                f2_unit(*fq[fi])
                                fi += 1
                while fi < len(fq):
                    f2_unit(*fq[fi])
                    fi += 1

            # ========= phase C: attention-value matmuls + residual =========
            with ExitStack() as phC:
                psY = phC.enter_context(tc.tile_pool(name="psY", bufs=4, space="PSUM"))
                p_xw = phC.enter_context(tc.tile_pool(name="p_xw", bufs=5))
                p_out = phC.enter_context(tc.tile_pool(name="p_out", bufs=4))

                ps_cs = psY.tile([1, C], dt.float32, name="ps_cs", tag="ps_cs")
                for pr in range(2):
                    for ip in range(2):
                        nc.tensor.matmul(ps_cs, ones_col,
                                         v_T[B - 1][pr][:, ip * C:(ip + 1) * C],
                                         start=(pr == 0 and ip == 0),
                                         stop=(pr == 1 and ip == 1))
                nc.vector.tensor_copy(cs_row, ps_cs)

                for jq in range(NJQ):
                    jsl = slice(jq * JQ, (jq + 1) * JQ)
                    x_wx = []
                    for b in range(B):
                        t = p_xw.tile([C + 1, JQ], dt.bfloat16, name="x_wx", tag="x_wx")
                        nc.sync.dma_start(t, x_ext.ap()[b][:, jsl])
                        x_wx.append(t)

                    ps_y = [psY.tile([C, JQ], dt.float32, name=f"ps_y{b}", tag="ps_y")
                            for b in range(B)]
                    for b in range(B):
                        nc.tensor.matmul(ps_y[b], sb_wT, x_wx[b],
                                         start=True, stop=False)
                    nc.tensor.matmul(ps_y[B - 1], cs_row, ones_row,
                                     start=False, stop=False)

                    def emit_copy(b):
                        out_sb = p_out.tile([C, JQ], dt.float32)
                        if b < 3:
                            nc.scalar.copy(out_sb, ps_y[b])
                        else:
                            nc.vector.tensor_copy(out_sb, ps_y[b])
                        nc.sync.dma_start(y_part.ap()[b][:, jsl], out_sb)

                    # b-major so each batch's PSUM drains while the next computes
                    for b in range(NB_E2):
                        for pr in range(2):
                            last = (pr == 1)
                            nc.tensor.matmul(
                                ps_y[b],
                                v1p[b][pr][:, :].rearrange("p (i c) -> p i c", i=2),
                                f1[b][pr][:, :].rearrange("p (i j) -> p i j", i=2)[:, :, jsl],
                                start=False, stop=False, perf_mode=pm_dr)
                            f2r = f2p[jq][b][:, pr * 2 * JQ:(pr + 1) * 2 * JQ] \
                                .rearrange("p (i j) -> p i j", i=2)
                            nc.tensor.matmul(
                                ps_y[b],
                                v_8[b][pr][:, :].rearrange("p (i c) -> p i c", i=2),
                                f2r,
                                start=False, stop=last, perf_mode=pm_dr)
                            nc.tensor.matmul(
                                ps_y[B - 1],
                                v8n[pr][:, :].rearrange("p (i c) -> p i c", i=2),
                                f2r,
                                start=False, stop=False, perf_mode=pm_dr)
                        emit_copy(b)
                    for pr in range(2):
                        nc.tensor.matmul(
                            ps_y[B - 1],
                            v1p[B - 1][pr][:, :].rearrange("p (i c) -> p i c", i=2),
                            f1[B - 1][pr][:, :].rearrange("p (i j) -> p i j", i=2)[:, :, jsl],
                            start=False, stop=(pr == 1), perf_mode=pm_dr)
                    emit_copy(B - 1)

    nc.compile()
    return nc


@functools.lru_cache(maxsize=1)
def _get_program():
    return _build_program()


def _prep_inputs(inputs):
    x = np.asarray(inputs["x"], np.float32).reshape(B, C, N)
    ones = np.ones((B, 1, N), np.float32)
    x_ext = np.concatenate([x, ones], axis=1).astype(BF16)          # [B,65,N]

    qw = np.asarray(inputs["qw"], np.float32)
    qb = np.asarray(inputs["qb"], np.float32)
    kw = np.asarray(inputs["kw"], np.float32)
    kb = np.asarray(inputs["kb"], np.float32)
    mw = np.asarray(inputs["mw"], np.float32)
    mb = np.asarray(inputs["mb"], np.float32)
    vw = np.asarray(inputs["vw"], np.float32)
    vb = np.asarray(inputs["vb"], np.float32)
    ww = np.asarray(inputs["ww"], np.float32)
    wb = np.asarray(inputs["wb"], np.float32)
    g = np.asarray(inputs["bn_gamma"], np.float32)
    be = np.asarray(inputs["bn_beta"], np.float32)
    rm = np.asarray(inputs["bn_rm"], np.float32)
    rv = np.asarray(inputs["bn_rv"], np.float32)

    qmT = np.zeros((C + 1, C + 1), np.float32)
    qmT[:C, :C] = qw.T
    qmT[C, :C] = qb
    qmT[:C, C] = mw[0]
    qmT[C, C] = mb[0]

    kT = np.concatenate([kw.T, kb[None, :]], axis=0)                # [65,64]

    vmT = np.zeros((C + 1, C + 1), np.float32)
    vmT[:C, :C] = vw.T * VSCALE
    vmT[C, :C] = vb * VSCALE
    vmT[:C, C] = mw[0]
    vmT[C, C] = mb[0]

    inv = g / np.sqrt(rv + EPS)
    wT = np.zeros((C + 1, C), np.float32)
    wT[:C, :] = (ww * inv[:, None]).T * (VSCALE / N_CORES)
    wT[C, :] = (wb * inv + be - rm * inv) * (VSCALE / N_CORES)

    sqf = np.zeros((1, K * 32), np.float32)
    for kk in range(K):
        sqf[0, kk * 32:(kk + 1) * 32] = 1.0 / math.sqrt(math.factorial(kk))

    common = {
        "qmT": qmT.astype(BF16),
        "kT": kT.astype(BF16),
        "vmT": vmT.astype(BF16),
        "wT": wT.astype(BF16),
        "sqf": sqf.astype(BF16),
    }
    in_maps = []
    for ic in range(N_CORES):
        m = dict(common)
        m["x_ext"] = np.ascontiguousarray(np.roll(x_ext, -ic * SL, axis=2))
        in_maps.append(m)
    return in_maps


def kernel(**inputs):
    from concourse.bass_utils import run_bass_kernel_spmd

    nc = _get_program()
    in_maps = _prep_inputs(inputs)
    res = run_bass_kernel_spmd(nc, in_maps, core_ids=list(range(N_CORES)))
    y = np.zeros((B, C, N), np.float32)
    for ic, r in enumerate(res.results):
        y += np.roll(r["y_part"], ic * SL, axis=2)
    y *= 1.0 / VSCALE
    return y.reshape(B, C, H, W)


if __name__ == "__main__":
    rng = np.random.default_rng(0)
    ins = {
        "x": rng.standard_normal((B, C, H, W), dtype=np.float32),
        "qw": rng.standard_normal((C, C), dtype=np.float32) * 0.05,
        "qb": rng.standard_normal((C,), dtype=np.float32) * 0.05,
        "kw": rng.standard_normal((C, C), dtype=np.float32) * 0.05,
        "kb": rng.standard_normal((C,), dtype=np.float32) * 0.05,
        "mw": rng.standard_normal((1, C), dtype=np.float32) * 0.05,
        "mb": rng.standard_normal((1,), dtype=np.float32) * 0.05,
        "vw": rng.standard_normal((C, C), dtype=np.float32) * 0.05,
        "vb": rng.standard_normal((C,), dtype=np.float32) * 0.05,
        "ww": rng.standard_normal((C, C), dtype=np.float32) * 0.05,
        "wb": rng.standard_normal((C,), dtype=np.float32) * 0.05,
        "bn_gamma": np.ones((C,), np.float32),
        "bn_beta": np.zeros((C,), np.float32),
        "bn_rm": np.zeros((C,), np.float32),
        "bn_rv": np.ones((C,), np.float32),
    }
    out = kernel(**ins)
    print("kernel output", out.shape, out.dtype, np.abs(out).mean())
